# revision 82
# baseline (speedup 1.0000x reference)
"""KDE log-density kernel for Trainium2, SPMD across 8 NeuronCores.

Computes log_p[m] = logsumexp_n(-scale * ||X[m] - svs[n]||^2) - log(N)
                    + (D/2) * log(scale/pi)

Sharding: 2-way over X rows x 4-way over svs rows (core i handles X
half i%2 against svs quarter i//2).  Each core returns raw partial sums
T[m] = sum_n exp(-scale*||x_m - s_n||^2) over its svs quarter; the host
sums the four quarters per query row and applies log(T) + C.

Per-core algorithm (fp8 DoubleRow pipeline):
  - psum accumulates 2*x.s via 2 DR matmuls per 512-col bank plus ONE
    K=1 DR inject matmul adding -(s2[n]-512): stationary pair (8, 0),
    moving a broadcast view of the fp8 row holding -(s2-512)/8.  The
    +512 recentering keeps the row accurate in fp8; the matching
    -scale*512 is folded into the exp bias.
  - s2 row: square-accumulate the f32 svs tiles at load time (DVE/ACT),
    scale to fp8 [128,16] per-partition, then a tiny DRAM roundtrip
    re-lays it out as a [1, 2048] row.
  - exp: per m-tile, 1536 psum cols get ACT Exp in-place (bias=
    -scale*(x2[m]+512), scale=scale, accum_out); the remaining 512 cols
    (separate 1-bank psum rotation) use a Schraudolph approx on DVE:
    one fused multiply-add writes int16 bf16-bit-patterns, then a
    bitcast-bf16 accumulate pass (4x DVE mode) sums them.
  - X/svs f32 loads are cast to fp8, echoed through DRAM, and
    DMA-xbar-transposed (u16-pair bitcast); the stationary operand is
    deinterleaved to the planar dual-fp8 ldweights layout (Pool).
  - svs pipelines in 512-row quarters ahead of X; X quarters stream
    during the main loop under tile_wait_until hints so their dep-free
    loads don't preempt the critical prologue DMAs.
"""

import sys

for _p in ("/opt/trn_rl_repo", "/opt/pypackages"):
    if _p not in sys.path:
        sys.path.insert(0, _p)

import numpy as np

M_FULL, N_FULL, D = 8192, 8192, 512
NCORES = 8
XSH, SSH = 2, 4           # X shards x svs shards
M_LOC = M_FULL // XSH     # 4096
N_LOC = N_FULL // SSH     # 2048
P = 128
MT = M_LOC // P           # 32 m-tiles per core
NCH = 512                 # psum bank of f32
NG = N_LOC                # single n-group
Q = 512                   # quarter rows for pipelined prologue
XQ = M_LOC // Q           # 8 X quarters

_CACHE = {}


def _build_nc():
    import concourse.mybir as mybir
    import concourse.tile as tile
    from concourse import bacc

    f32 = mybir.dt.float32
    fp8 = mybir.dt.float8e4
    u16 = mybir.dt.uint16
    AF = mybir.ActivationFunctionType
    ALU = mybir.AluOpType

    DR = mybir.MatmulPerfMode.DoubleRow

    nc = bacc.Bacc(None, target_bir_lowering=False, debug=True)
    Xd = nc.declare_dram_parameter("X", [M_LOC, D], f32, isOutput=False)
    Sd = nc.declare_dram_parameter("svs", [N_LOC, D], f32, isOutput=False)
    scd = nc.declare_dram_parameter("scale", [1], f32, isOutput=False)
    outd = nc.declare_dram_parameter("out", [M_LOC], f32, isOutput=True)

    def drpair(ap2d, n0, ncols):
        # fp8 [p, 2*cols] packed-pair slice -> DoubleRow [p, 2, ncols]
        return ap2d[:, 2 * n0:2 * (n0 + ncols)].rearrange(
            "p (n two) -> p two n", two=2
        )

    with tile.TileContext(nc) as tc:
        with (
            tc.tile_pool(name="const", bufs=1) as cp,
            tc.tile_pool(name="stage", bufs=2) as stp,
            tc.tile_pool(name="s8", bufs=2) as s8p,
            tc.tile_pool(name="small", bufs=4) as sp,
            tc.tile_pool(name="wpsum", bufs=2, space="PSUM") as pp,
            tc.tile_pool(name="ppsum", bufs=2, space="PSUM") as pb,
            tc.tile_pool(name="dram", bufs=1, space="DRAM") as dp,
        ):
            # ---------- constants ----------
            scale_bc = cp.tile([P, 1], f32)
            nc.sync.dma_start(scale_bc[:], scd[None, :].to_broadcast((P, 1)))
            negscale = cp.tile([P, 1], f32)
            nc.scalar.mul(negscale[:], scale_bc[:], -1.0)
            # Schraudolph bf16-bits exp: bits = pw*(128*log2e*scale) + B[m]
            acst = cp.tile([P, 1], f32)
            nc.vector.tensor_scalar_mul(acst[:], scale_bc[:], 184.66496)
            ones8 = cp.tile([1, 2, P], fp8)
            nc.gpsimd.memset(ones8[:, 0, :], 8.0)
            nc.gpsimd.memset(ones8[:, 1, :], 0.0)


            # resident tensors
            x8 = cp.tile([P, MT, D], fp8)
            xT8 = cp.tile([P, 2, M_LOC * 2], fp8)   # packed pairs (m, lane)
            xT8p = cp.tile([P, 2, 2, M_LOC], fp8)   # planar [p, K, lane, m]
            x2_all = cp.tile([P, MT], f32)
            x2sc = cp.tile([P, MT], f32)
            ball = cp.tile([P, MT], f32)            # Schraudolph per-m offset
            svsT8 = cp.tile([P, 2, NG * 2], fp8)
            s2pp8 = cp.tile([P, NG // P], fp8)      # -(s2-512)/8, [p, j]
            s2row8 = cp.tile([1, 2 * NG], fp8)      # interleaved (v, 0) pairs
            s2tmp = cp.tile([P, NG // P], f32)      # +s2 per-partition
            partials = cp.tile([P, MT], f32)
            part_d = cp.tile([P, MT], f32)
            part_p = cp.tile([P, MT], f32)

            x8d = dp.tile([M_LOC, D], fp8)
            s8d = dp.tile([N_LOC, D], fp8)
            s2d8 = dp.tile([NG], fp8)

            nc.gpsimd.memset(s2row8[:], 0.0)

            def lhsT(t, K):
                # planar pair view of transposed X for ldweights
                return xT8p[:, K, :, P * t:P * (t + 1)]

            def injrow(n0, ncols):
                # K=1 moving operand [1, 2, ncols] from interleaved (v, 0)
                # pairs; stationary pair is (8, 0)
                return s2row8[0:1, 2 * n0:2 * (n0 + ncols)].rearrange(
                    "p (n two) -> p two n", two=2
                )

            # ---------- svs pipeline (quarters of 512 rows) ----------
            sv_stage = {}

            def sv_load(q):
                svst = stp.tile([P, Q // P, D], f32, tag="svst", bufs=4)
                sv_stage[("f", q)] = svst
                nc.sync.dma_start(
                    svst[:],
                    Sd[Q * q:Q * (q + 1), :].rearrange("(j p) d -> p j d", p=P),
                )

            def sv_sq(q, eng):
                # s2 accumulate per j-tile from the staged f32 quarter
                svst = sv_stage.pop(("f", q))
                for j in range(Q // P):
                    jj = q * (Q // P) + j
                    if eng is nc.scalar:
                        ssq = sp.tile([P, D], f32, tag="ssq")
                        nc.scalar.activation(
                            ssq[:], svst[:, j, :], AF.Square,
                            accum_out=s2tmp[:, jj:jj + 1],
                        )
                    else:
                        ssq = sp.tile([P, D], fp8, tag="ssq8")
                        nc.vector.scalar_tensor_tensor(
                            ssq[:], svst[:, j, :], 1.0, svst[:, j, :],
                            ALU.mult, ALU.mult,
                            accum_out=s2tmp[:, jj:jj + 1],
                        )
                # -(s2 - 512)/8 in fp8
                j0 = q * (Q // P)
                nc.vector.tensor_scalar(
                    s2pp8[:, j0:j0 + Q // P], s2tmp[:, j0:j0 + Q // P],
                    -0.125, 64.0, ALU.mult, ALU.add,
                )

            def sv_cast(q, eng):
                svst = sv_stage[("f", q)]
                sv8 = s8p.tile([P, Q // P, D], fp8, tag="sv8", bufs=2)
                sv_stage[("8", q)] = sv8
                eng.tensor_scalar_mul(sv8[:], svst[:], 2.0)

            def sv_store(q):
                sv8 = sv_stage.pop(("8", q))
                nc.sync.dma_start(
                    s8d[Q * q:Q * (q + 1), :].rearrange(
                        "(j p) d -> p j d", p=P),
                    sv8[:],
                )

            def sv_transp(q, dmaq):
                dmaq.dma_start_transpose(
                    svsT8.bitcast(u16)[:, :, Q * q:Q * (q + 1)],
                    s8d.bitcast(u16)[Q * q:Q * (q + 1), :],
                )

            def s2_roundtrip():
                nc.scalar.dma_start(
                    s2d8[:].rearrange("(j p) -> p j", p=P), s2pp8[:]
                )
                # strided 1-byte writes land the row in the even pair lanes
                nc.scalar.dma_start(
                    s2row8[0:1, :].rearrange("p (n two) -> p n two", two=2)[
                        :, :, 0:1],
                    s2d8[None, :, None],
                )

            # ---------- X pipeline (quarters of 512 rows / 4 m-tiles) ----
            x_stage = {}

            def x_load(q):
                xst = stp.tile([P, 4, D], f32, tag="xst", bufs=3)
                x_stage[q] = xst
                nc.sync.dma_start(
                    xst[:],
                    Xd[Q * q:Q * (q + 1), :].rearrange(
                        "(t p) d -> p t d", p=P),
                )

            def x_cast(q, eng):
                eng.tensor_copy(
                    x8[:, 4 * q:4 * (q + 1), :], x_stage[q][:]
                )

            def x_store(q, dmaq=None):
                (dmaq or nc.sync).dma_start(
                    x8d[Q * q:Q * (q + 1), :].rearrange(
                        "(t p) d -> p t d", p=P),
                    x8[:, 4 * q:4 * (q + 1), :],
                )

            def x_transp(q):
                nc.sync.dma_start_transpose(
                    xT8.bitcast(u16)[:, :, Q * q:Q * (q + 1)],
                    x8d.bitcast(u16)[Q * q:Q * (q + 1), :],
                )

            def x_deint(q, eng, eng2=None):
                # planar stationary layout for the dual-fp8 ldweights ISA;
                # first tile's 128 cols split out so its matmuls start early
                for K in range(2):
                    for i in range(2):
                        e = eng if (eng2 is None or K == 0) else eng2
                        e.tensor_copy(
                            xT8p[:, K, i, Q * q:Q * q + P],
                            xT8[:, K, 2 * Q * q + i:2 * (Q * q + P):2],
                        )
                for K in range(2):
                    for i in range(2):
                        e = eng if (eng2 is None or K == 0) else eng2
                        e.tensor_copy(
                            xT8p[:, K, i, Q * q + P:Q * (q + 1)],
                            xT8[:, K, 2 * (Q * q + P) + i:2 * Q * (q + 1):2],
                        )

            def x_sq(q, eng):
                xst = x_stage[q]
                for t in range(4):
                    tt = 4 * q + t
                    if eng is nc.scalar:
                        xsq = sp.tile([P, D], f32, tag="xsq")
                        nc.scalar.activation(
                            xsq[:], xst[:, t, :], AF.Square,
                            accum_out=x2_all[:, tt:tt + 1],
                        )
                    else:
                        xsq = sp.tile([P, D], fp8, tag="xsq8")
                        nc.vector.scalar_tensor_tensor(
                            xsq[:], xst[:, t, :], 1.0, xst[:, t, :],
                            ALU.mult, ALU.mult,
                            accum_out=x2_all[:, tt:tt + 1],
                        )
                x_stage.pop(q)

            def x2sc_piece(q, eng=None):
                eng = eng or nc.vector
                # x2sc = (x2 + 512) * (-scale)
                eng.tensor_scalar(
                    x2sc[:, 4 * q:4 * (q + 1)],
                    x2_all[:, 4 * q:4 * (q + 1)],
                    512.0, negscale[:], ALU.add, ALU.mult,
                )
                # bf16-bits offset: 128*log2e*x2sc + 128*126.94269
                eng.tensor_scalar(
                    ball[:, 4 * q:4 * (q + 1)],
                    x2sc[:, 4 * q:4 * (q + 1)],
                    184.66496, 16248.665, ALU.mult, ALU.add,
                )

            # ---------- main loop ----------
            ACOLS = 1536              # exp on ACT (3 psum banks)
            PCOLS = NG - ACOLS        # Schraudolph on DVE (1 bank)
            i16 = mybir.dt.int16
            bf16 = mybir.dt.bfloat16

            def schraudolph(src, t, accum):
                # pass 1 (DVE, psum-capable): bf16 bit pattern as int16
                seg = sp.tile([P, PCOLS], i16, tag="segp", bufs=2)
                nc.vector.tensor_scalar(
                    seg[:], src[:],
                    acst[:], ball[:, t:t + 1], ALU.mult, ALU.add,
                )
                # pass 2 (DVE, all-SBUF 2-byte -> 4x mode): sum the bf16s
                segb = seg.bitcast(bf16)
                nc.vector.tensor_scalar(
                    segb[:], segb[:], 1.0, 0.0, ALU.mult, ALU.add,
                    accum_out=accum[:, t:t + 1],
                )

            def bank_mms(bank, t, n0):
                for K in range(2):
                    nc.tensor.matmul(
                        bank,
                        lhsT(t, K),
                        drpair(svsT8[:, K, :], n0, NCH),
                        start=(K == 0),
                        stop=False,
                        perf_mode=DR,
                    )
                nc.tensor.matmul(
                    bank,
                    ones8[:],
                    injrow(n0, NCH),
                    start=False,
                    stop=True,
                    perf_mode=DR,
                )

            def mm_tile(t):
                pw = pp.tile([P, ACOLS], f32, tag="pw")
                pq = pb.tile([P, PCOLS], f32, tag="pq")
                for c in range(ACOLS // NCH):
                    bank_mms(pw[:, c * NCH:(c + 1) * NCH], t, c * NCH)
                bank_mms(pq[:], t, ACOLS)
                nc.scalar.activation(
                    pw[:], pw[:], AF.Exp,
                    bias=x2sc[:, t:t + 1], scale=scale_bc[:],
                    accum_out=partials[:, t:t + 1],
                )
                schraudolph(pq, t, part_p)

            # ---------- emission ----------
            # X q0 then svs quarters (they gate the first exp); the echo
            # ladder issues stores then transposes so the serial DMA device
            # stays busy without head-of-line bubbles
            x_load(0)
            for q in range(4):
                sv_load(q)
            x_cast(0, nc.vector)
            for q in range(4):
                sv_cast(q, nc.vector)
            x_store(0)
            for q in range(4):
                sv_store(q)
            x_transp(0)
            for q in range(4):
                sv_transp(q, nc.sync if q < 2 else nc.scalar)
            x_deint(0, nc.vector)
            for q in range(4):
                sv_sq(q, nc.vector if q < 2 else nc.scalar)
            s2_roundtrip()
            x_sq(0, nc.scalar)
            x2sc_piece(0)

            # remaining X quarters stream behind; wait hints keep their
            # dep-free loads from jumping ahead of the critical prologue
            # DMAs on the serial DMA device
            for q in range(1, XQ):
                with tc.tile_wait_until(0.018 + 0.006 * (q - 1)):
                    x_load(q)
                x_cast(q, nc.vector)
                x_store(q)
                x_transp(q)
                x_deint(q, nc.gpsimd)
                x_sq(q, nc.vector)
                x2sc_piece(q)

            H = MT // 2
            for t in range(MT):
                mm_tile(t)
                if t == H - 1 or t == MT - 1:
                    lo = 0 if t < H else H
                    nc.vector.tensor_tensor(
                        partials[:, lo:lo + H],
                        partials[:, lo:lo + H],
                        part_p[:, lo:lo + H],
                        ALU.add,
                    )
                    nc.sync.dma_start(
                        outd[Q * lo // 4:Q * (lo + H) // 4].rearrange(
                            "(t p) -> p t", p=P),
                        partials[:, lo:lo + H],
                    )

    nc.finalize()
    return nc


def kernel(X: np.ndarray, svs: np.ndarray, scale: np.ndarray) -> np.ndarray:
    from concourse.bass_utils import run_bass_kernel_spmd

    if "nc" not in _CACHE:
        _CACHE["nc"] = _build_nc()
    nc = _CACHE["nc"]

    X = np.ascontiguousarray(X, dtype=np.float32)
    svs = np.ascontiguousarray(svs, dtype=np.float32)
    sc = np.asarray(scale, dtype=np.float32).reshape(1)

    in_maps = [
        {
            "X": X[(i % XSH) * M_LOC:(i % XSH + 1) * M_LOC],
            "svs": svs[(i // XSH) * N_LOC:(i // XSH + 1) * N_LOC],
            "scale": sc,
        }
        for i in range(NCORES)
    ]
    res = run_bass_kernel_spmd(nc, in_maps, core_ids=list(range(NCORES)))
    T = [r["out"].reshape(M_LOC).astype(np.float64) for r in res.results]
    C = float(-np.log(N_FULL) + (D / 2) * np.log(float(sc[0]) / np.pi))
    out = np.concatenate(
        [np.log(sum(T[h + XSH * s] for s in range(SSH))) + C
         for h in range(XSH)]
    )
    return out.astype(np.float32)


# revision 86
# speedup vs baseline: 1.0207x; 1.0207x over previous
"""KDE log-density kernel for Trainium2, SPMD across 8 NeuronCores.

Computes log_p[m] = logsumexp_n(-scale * ||X[m] - svs[n]||^2) - log(N)
                    + (D/2) * log(scale/pi)

Sharding: 2-way over X rows x 4-way over svs rows (core i handles X
half i%2 against svs quarter i//2).  Each core returns raw partial sums
T[m] = sum_n exp(-scale*||x_m - s_n||^2) over its svs quarter; the host
sums the four quarters per query row and applies log(T) + C.

Per-core algorithm (fp8 DoubleRow pipeline):
  - psum accumulates 2*x.s via 2 DR matmuls per 512-col bank plus ONE
    K=1 DR inject matmul adding -(s2[n]-512): stationary pair (8, 0),
    moving a broadcast view of the fp8 row holding -(s2-512)/8.  The
    +512 recentering keeps the row accurate in fp8; the matching
    -scale*512 is folded into the exp bias.
  - s2 row: square-accumulate the f32 svs tiles at load time (DVE/ACT),
    scale to fp8 [128,16] per-partition, then a tiny DRAM roundtrip
    re-lays it out as a [1, 2048] row.
  - exp: per m-tile, 1536 psum cols get ACT Exp in-place (bias=
    -scale*(x2[m]+512), scale=scale, accum_out); the remaining 512 cols
    (separate 1-bank psum rotation) use a Schraudolph approx on DVE:
    one fused multiply-add writes int16 bf16-bit-patterns, then a
    bitcast-bf16 accumulate pass (4x DVE mode) sums them.
  - X/svs f32 loads are cast to fp8, echoed through DRAM, and
    DMA-xbar-transposed (u16-pair bitcast); the stationary operand is
    deinterleaved to the planar dual-fp8 ldweights layout (Pool).
  - svs pipelines in 512-row quarters ahead of X; X quarters stream
    during the main loop under tile_wait_until hints so their dep-free
    loads don't preempt the critical prologue DMAs.
"""

import sys

for _p in ("/opt/trn_rl_repo", "/opt/pypackages"):
    if _p not in sys.path:
        sys.path.insert(0, _p)

import numpy as np

M_FULL, N_FULL, D = 8192, 8192, 512
NCORES = 8
XSH, SSH = 2, 4           # X shards x svs shards
M_LOC = M_FULL // XSH     # 4096
N_LOC = N_FULL // SSH     # 2048
P = 128
MT = M_LOC // P           # 32 m-tiles per core
NCH = 512                 # psum bank of f32
NG = N_LOC                # single n-group
Q = 512                   # quarter rows for pipelined prologue
XQ = M_LOC // Q           # 8 X quarters

_CACHE = {}


def _build_nc():
    import concourse.mybir as mybir
    import concourse.tile as tile
    from concourse import bacc

    f32 = mybir.dt.float32
    fp8 = mybir.dt.float8e4
    u16 = mybir.dt.uint16
    AF = mybir.ActivationFunctionType
    ALU = mybir.AluOpType

    DR = mybir.MatmulPerfMode.DoubleRow

    nc = bacc.Bacc(None, target_bir_lowering=False, debug=True)
    Xd = nc.declare_dram_parameter("X", [M_LOC, D], f32, isOutput=False)
    Sd = nc.declare_dram_parameter("svs", [N_LOC, D], f32, isOutput=False)
    scd = nc.declare_dram_parameter("scale", [1], f32, isOutput=False)
    outd = nc.declare_dram_parameter("out", [M_LOC], f32, isOutput=True)

    def drpair(ap2d, n0, ncols):
        # fp8 [p, 2*cols] packed-pair slice -> DoubleRow [p, 2, ncols]
        return ap2d[:, 2 * n0:2 * (n0 + ncols)].rearrange(
            "p (n two) -> p two n", two=2
        )

    with tile.TileContext(nc) as tc:
        with (
            tc.tile_pool(name="const", bufs=1) as cp,
            tc.tile_pool(name="stage", bufs=2) as stp,
            tc.tile_pool(name="s8", bufs=2) as s8p,
            tc.tile_pool(name="small", bufs=4) as sp,
            tc.tile_pool(name="wpsum", bufs=2, space="PSUM") as pp,
            tc.tile_pool(name="ppsum", bufs=2, space="PSUM") as pb,
            tc.tile_pool(name="dram", bufs=1, space="DRAM") as dp,
        ):
            # ---------- constants ----------
            scale_bc = cp.tile([P, 1], f32)
            nc.sync.dma_start(scale_bc[:], scd[None, :].to_broadcast((P, 1)))
            negscale = cp.tile([P, 1], f32)
            nc.scalar.mul(negscale[:], scale_bc[:], -1.0)
            # Schraudolph bf16-bits exp: bits = pw*(128*log2e*scale) + B[m]
            acst = cp.tile([P, 1], f32)
            nc.vector.tensor_scalar_mul(acst[:], scale_bc[:], 184.66496)
            ones8 = cp.tile([1, 2, P], fp8)
            nc.gpsimd.memset(ones8[:, 0, :], 8.0)
            nc.gpsimd.memset(ones8[:, 1, :], 0.0)


            # resident tensors
            x8 = cp.tile([P, MT, D], fp8)
            xT8 = cp.tile([P, 2, M_LOC * 2], fp8)   # packed pairs (m, lane)
            xT8p = cp.tile([P, 2, 2, M_LOC], fp8)   # planar [p, K, lane, m]
            x2_all = cp.tile([P, MT], f32)
            x2sc = cp.tile([P, MT], f32)
            ball = cp.tile([P, MT], f32)            # Schraudolph per-m offset
            svsT8 = cp.tile([P, 2, NG * 2], fp8)
            s2pp8 = cp.tile([P, NG // P], fp8)      # -(s2-512)/8, [p, j]
            s2row8 = cp.tile([1, 2 * NG], fp8)      # interleaved (v, 0) pairs
            s2tmp = cp.tile([P, NG // P], f32)      # +s2 per-partition
            partials = cp.tile([P, MT], f32)
            part_d = cp.tile([P, MT], f32)
            part_p = cp.tile([P, MT], f32)

            x8d = dp.tile([M_LOC, D], fp8)
            s8d = dp.tile([N_LOC, D], fp8)
            s2d8 = dp.tile([NG], fp8)

            nc.gpsimd.memset(s2row8[:], 0.0)

            def lhsT(t, K):
                # planar pair view of transposed X for ldweights
                return xT8p[:, K, :, P * t:P * (t + 1)]

            def injrow(n0, ncols):
                # K=1 moving operand [1, 2, ncols] from interleaved (v, 0)
                # pairs; stationary pair is (8, 0)
                return s2row8[0:1, 2 * n0:2 * (n0 + ncols)].rearrange(
                    "p (n two) -> p two n", two=2
                )

            # ---------- svs pipeline (quarters of 512 rows) ----------
            sv_stage = {}

            def sv_load(q):
                svst = stp.tile([P, Q // P, D], f32, tag="svst", bufs=4)
                sv_stage[("f", q)] = svst
                nc.sync.dma_start(
                    svst[:],
                    Sd[Q * q:Q * (q + 1), :].rearrange("(j p) d -> p j d", p=P),
                )

            def sv_sq(q, eng):
                # s2 accumulate per j-tile from the staged f32 quarter
                svst = sv_stage.pop(("f", q))
                for j in range(Q // P):
                    jj = q * (Q // P) + j
                    if eng is nc.scalar:
                        ssq = sp.tile([P, D], f32, tag="ssq")
                        nc.scalar.activation(
                            ssq[:], svst[:, j, :], AF.Square,
                            accum_out=s2tmp[:, jj:jj + 1],
                        )
                    else:
                        ssq = sp.tile([P, D], fp8, tag="ssq8")
                        nc.vector.scalar_tensor_tensor(
                            ssq[:], svst[:, j, :], 1.0, svst[:, j, :],
                            ALU.mult, ALU.mult,
                            accum_out=s2tmp[:, jj:jj + 1],
                        )
                # -(s2 - 512)/8 in fp8
                j0 = q * (Q // P)
                nc.vector.tensor_scalar(
                    s2pp8[:, j0:j0 + Q // P], s2tmp[:, j0:j0 + Q // P],
                    -0.125, 64.0, ALU.mult, ALU.add,
                )

            def sv_cast(q, eng):
                svst = sv_stage[("f", q)]
                sv8 = s8p.tile([P, Q // P, D], fp8, tag="sv8", bufs=2)
                sv_stage[("8", q)] = sv8
                eng.tensor_scalar_mul(sv8[:], svst[:], 2.0)

            def sv_store(q):
                sv8 = sv_stage.pop(("8", q))
                nc.sync.dma_start(
                    s8d[Q * q:Q * (q + 1), :].rearrange(
                        "(j p) d -> p j d", p=P),
                    sv8[:],
                )

            def sv_transp(q, dmaq):
                dmaq.dma_start_transpose(
                    svsT8.bitcast(u16)[:, :, Q * q:Q * (q + 1)],
                    s8d.bitcast(u16)[Q * q:Q * (q + 1), :],
                )

            def s2_roundtrip():
                nc.scalar.dma_start(
                    s2d8[:].rearrange("(j p) -> p j", p=P), s2pp8[:]
                )
                # strided 1-byte writes land the row in the even pair lanes
                nc.scalar.dma_start(
                    s2row8[0:1, :].rearrange("p (n two) -> p n two", two=2)[
                        :, :, 0:1],
                    s2d8[None, :, None],
                )

            # ---------- X pipeline (quarters of 512 rows / 4 m-tiles) ----
            x_stage = {}

            def x_load(q):
                xst = stp.tile([P, 4, D], f32, tag="xst", bufs=3)
                x_stage[q] = xst
                nc.sync.dma_start(
                    xst[:],
                    Xd[Q * q:Q * (q + 1), :].rearrange(
                        "(t p) d -> p t d", p=P),
                )

            def x_cast(q, eng):
                eng.tensor_copy(
                    x8[:, 4 * q:4 * (q + 1), :], x_stage[q][:]
                )

            def x_store(q, dmaq=None):
                (dmaq or nc.sync).dma_start(
                    x8d[Q * q:Q * (q + 1), :].rearrange(
                        "(t p) d -> p t d", p=P),
                    x8[:, 4 * q:4 * (q + 1), :],
                )

            def x_transp(q, dmaq=None):
                (dmaq or nc.sync).dma_start_transpose(
                    xT8.bitcast(u16)[:, :, Q * q:Q * (q + 1)],
                    x8d.bitcast(u16)[Q * q:Q * (q + 1), :],
                )

            def x_deint(q, eng, eng2=None):
                # planar stationary layout for the dual-fp8 ldweights ISA;
                # first tile's 128 cols split out so its matmuls start early
                for K in range(2):
                    for i in range(2):
                        e = eng if (eng2 is None or K == 0) else eng2
                        e.tensor_copy(
                            xT8p[:, K, i, Q * q:Q * q + P],
                            xT8[:, K, 2 * Q * q + i:2 * (Q * q + P):2],
                        )
                for K in range(2):
                    for i in range(2):
                        e = eng if (eng2 is None or K == 0) else eng2
                        e.tensor_copy(
                            xT8p[:, K, i, Q * q + P:Q * (q + 1)],
                            xT8[:, K, 2 * (Q * q + P) + i:2 * Q * (q + 1):2],
                        )

            def x_sq(q, eng):
                xst = x_stage[q]
                for t in range(4):
                    tt = 4 * q + t
                    if eng is nc.scalar:
                        xsq = sp.tile([P, D], f32, tag="xsq")
                        nc.scalar.activation(
                            xsq[:], xst[:, t, :], AF.Square,
                            accum_out=x2_all[:, tt:tt + 1],
                        )
                    else:
                        xsq = sp.tile([P, D], fp8, tag="xsq8")
                        nc.vector.scalar_tensor_tensor(
                            xsq[:], xst[:, t, :], 1.0, xst[:, t, :],
                            ALU.mult, ALU.mult,
                            accum_out=x2_all[:, tt:tt + 1],
                        )
                x_stage.pop(q)

            def x2sc_piece(q, eng=None):
                eng = eng or nc.vector
                # x2sc = (x2 + 512) * (-scale)
                eng.tensor_scalar(
                    x2sc[:, 4 * q:4 * (q + 1)],
                    x2_all[:, 4 * q:4 * (q + 1)],
                    512.0, negscale[:], ALU.add, ALU.mult,
                )
                # bf16-bits offset: 128*log2e*x2sc + 128*126.94269
                eng.tensor_scalar(
                    ball[:, 4 * q:4 * (q + 1)],
                    x2sc[:, 4 * q:4 * (q + 1)],
                    184.66496, 16248.665, ALU.mult, ALU.add,
                )

            # ---------- main loop ----------
            ACOLS = 1536              # exp on ACT (3 psum banks)
            PCOLS = NG - ACOLS        # Schraudolph on DVE (1 bank)
            i16 = mybir.dt.int16
            bf16 = mybir.dt.bfloat16

            def schraudolph(src, t, accum):
                # pass 1 (DVE, psum-capable): bf16 bit pattern as int16
                seg = sp.tile([P, PCOLS], i16, tag="segp", bufs=2)
                nc.vector.tensor_scalar(
                    seg[:], src[:],
                    acst[:], ball[:, t:t + 1], ALU.mult, ALU.add,
                )
                # pass 2 (DVE, all-SBUF 2-byte -> 4x mode): sum the bf16s
                segb = seg.bitcast(bf16)
                nc.vector.tensor_scalar(
                    segb[:], segb[:], 1.0, 0.0, ALU.mult, ALU.add,
                    accum_out=accum[:, t:t + 1],
                )

            def bank_mms(bank, t, n0):
                for K in range(2):
                    nc.tensor.matmul(
                        bank,
                        lhsT(t, K),
                        drpair(svsT8[:, K, :], n0, NCH),
                        start=(K == 0),
                        stop=False,
                        perf_mode=DR,
                    )
                nc.tensor.matmul(
                    bank,
                    ones8[:],
                    injrow(n0, NCH),
                    start=False,
                    stop=True,
                    perf_mode=DR,
                )

            def mm_tile(t):
                pw = pp.tile([P, ACOLS], f32, tag="pw")
                pq = pb.tile([P, PCOLS], f32, tag="pq")
                for c in range(ACOLS // NCH):
                    bank_mms(pw[:, c * NCH:(c + 1) * NCH], t, c * NCH)
                bank_mms(pq[:], t, ACOLS)
                nc.scalar.activation(
                    pw[:], pw[:], AF.Exp,
                    bias=x2sc[:, t:t + 1], scale=scale_bc[:],
                    accum_out=partials[:, t:t + 1],
                )
                schraudolph(pq, t, part_p)

            # ---------- emission ----------
            # X q0 then svs quarters (they gate the first exp); the echo
            # ladder issues stores then transposes so the serial DMA device
            # stays busy without head-of-line bubbles
            x_load(0)
            for q in range(4):
                sv_load(q)
            x_cast(0, nc.vector)
            for q in range(4):
                sv_cast(q, nc.vector)
            x_store(0)
            for q in range(4):
                sv_store(q)
            x_transp(0)
            for q in range(4):
                sv_transp(q, nc.sync if q < 2 else nc.scalar)
            x_deint(0, nc.vector)
            for q in range(4):
                sv_sq(q, nc.vector if q < 2 else nc.scalar)
            s2_roundtrip()
            x_sq(0, nc.scalar)
            x2sc_piece(0)

            # remaining X quarters stream behind; wait hints keep their
            # dep-free loads from jumping ahead of the critical prologue
            # DMAs on the serial DMA device
            for q in range(1, XQ):
                with tc.tile_wait_until(0.018 + 0.006 * (q - 1)):
                    x_load(q)
                x_cast(q, nc.vector)
                dq = nc.scalar if q == 1 else nc.sync
                x_store(q, dq)
                x_transp(q, dq)
                x_deint(q, nc.gpsimd)
                x_sq(q, nc.vector)
                x2sc_piece(q)

            H = MT // 2
            for t in range(MT):
                mm_tile(t)
                if t == H - 1 or t == MT - 1:
                    lo = 0 if t < H else H
                    nc.vector.tensor_tensor(
                        partials[:, lo:lo + H],
                        partials[:, lo:lo + H],
                        part_p[:, lo:lo + H],
                        ALU.add,
                    )
                    nc.sync.dma_start(
                        outd[Q * lo // 4:Q * (lo + H) // 4].rearrange(
                            "(t p) -> p t", p=P),
                        partials[:, lo:lo + H],
                    )

    nc.finalize()
    return nc


def kernel(X: np.ndarray, svs: np.ndarray, scale: np.ndarray) -> np.ndarray:
    from concourse.bass_utils import run_bass_kernel_spmd

    if "nc" not in _CACHE:
        _CACHE["nc"] = _build_nc()
    nc = _CACHE["nc"]

    X = np.ascontiguousarray(X, dtype=np.float32)
    svs = np.ascontiguousarray(svs, dtype=np.float32)
    sc = np.asarray(scale, dtype=np.float32).reshape(1)

    in_maps = [
        {
            "X": X[(i % XSH) * M_LOC:(i % XSH + 1) * M_LOC],
            "svs": svs[(i // XSH) * N_LOC:(i // XSH + 1) * N_LOC],
            "scale": sc,
        }
        for i in range(NCORES)
    ]
    res = run_bass_kernel_spmd(nc, in_maps, core_ids=list(range(NCORES)))
    T = [r["out"].reshape(M_LOC).astype(np.float64) for r in res.results]
    C = float(-np.log(N_FULL) + (D / 2) * np.log(float(sc[0]) / np.pi))
    out = np.concatenate(
        [np.log(sum(T[h + XSH * s] for s in range(SSH))) + C
         for h in range(XSH)]
    )
    return out.astype(np.float32)
